# revision 76
# baseline (speedup 1.0000x reference)
"""Trainium2 Bass kernel for nn_DigitCap (sparse_attention).

Math note: the reference's softmax is over a size-1 axis, so C == 1 exactly
and the N x N attention matrix A is dead code.  The computation collapses to

    S[b,d,i]  = sum_{n,j} (1 + B[d,n]) * W[d,n,i,j] * U[b,n,j]
    out[b,d,:] = (1 - exp(-|S|)) * S / (|S| + 1e-7)

Sharding: split by digit capsule d (2 of 10 per core, zero-padded to a
uniform 2 so the SPMD program is identical on all 8 cores).

Key implementation notes:
  * bf16 inputs (tolerance is 2e-2; bf16 keeps us ~1e-2): halves HBM
    traffic and runs the PE at 1 cycle/row instead of fp32's 4.
  * six input DMAs balanced over the three DMA queues (each dma_start
    costs ~565ns sequencer time + ~900ns completion-semaphore
    propagation; the measured fabric tops out ~190GB/s aggregate,
    ~95-125GB/s per queue): SP carries W_hi+B then W_lo, ACT and Pool
    interleave the four U quarters so consecutive-consumed pieces never
    queue behind each other (PE eats a quarter in ~0.5us; one queue
    delivers only every ~1.1us).
  * (1+B)*W runs per W-quarter on DVE so only the first ~400ns scale
    gates the PE start.
  * single PSUM accumulation group across all 32 matmuls.
  * epilogue needs sqrt AND exp, which live in different ACT tables
    (1283ns load each).  sqrt(x) = exp(0.5*ln(x)) lets the whole
    epilogue use the one natural_log+exp table, pre-warmed during the
    DMA phase; 1/norm = exp(-0.5*ln(ss)) replaces eps-add + reciprocal.
    ln(0) = -inf on the pad capsules -> NaN outputs there; host discards.
  * DVE does NOT interlock same-engine read-after-write (8-deep exec
    queue reads stale operands), so the finisher is two fused
    scalar_tensor_tensor ops with one self-semaphore between them:
        fac' = (et - 1) * rn ; out = (-S) * fac'
"""

import numpy as np
import ml_dtypes
from contextlib import ExitStack

import concourse.bass as bass
import concourse.mybir as mybir
from concourse.bass_utils import run_bass_kernel_spmd

F32 = mybir.dt.float32
BF16 = mybir.dt.bfloat16
AF = mybir.ActivationFunctionType
ALU = mybir.AluOpType
P = 128
D, DD, N, DP = 10, 16, 512, 8     # digit caps, digit dim, primary caps, primary dim
K = N * DP                         # 4096 contraction
NCHUNK = K // P                    # 32 chunks of 128 contraction rows
NCORES = 8
BFULL = 64
DC = 2                             # d's per core (8*2 = 16 slots >= 10 real)
DIC = DC * DD                      # 32 output cols per core
HC = NCHUNK // 2                   # 16
QC = NCHUNK // 4                   # 8 chunks per U quarter
WHALF = HC * DIC                   # 512 cols per W half
BCOLS = NCHUNK * DC                # 64
UCOLS = NCHUNK * BFULL             # 2048
# wb DRAM/SBUF layout: [W_hi (chunks 16..31) | B (all) | W_lo (chunks 0..15)]
WB_COLS = WHALF + BCOLS + WHALF    # 1088
OFF_B = WHALF                      # 512
OFF_WLO = WHALF + BCOLS            # 576


def _wcol(c):
    """start col of chunk c's W block inside wb"""
    return (c - HC) * DIC if c >= HC else OFF_WLO + c * DIC


def build_raw():
    nc = bass.Bass()
    u_t = nc.dram_tensor("u_t", [P, UCOLS], BF16, kind="ExternalInput")
    wb_t = nc.dram_tensor("wb_t", [P, WB_COLS], BF16, kind="ExternalInput")
    out = nc.dram_tensor("out", [BFULL, DIC], F32, kind="ExternalOutput")

    with ExitStack() as ctx:
        u_all = ctx.enter_context(nc.sbuf_tensor("u_all", [P, UCOLS], BF16))
        wb = ctx.enter_context(nc.sbuf_tensor("wb", [P, WB_COLS], BF16))
        ps = ctx.enter_context(nc.psum_tensor("ps", [BFULL, DIC], F32))
        sq = ctx.enter_context(nc.sbuf_tensor("sq", [BFULL, DIC], F32))
        ss = ctx.enter_context(nc.sbuf_tensor("ss", [BFULL, DC], F32))
        lss = ctx.enter_context(nc.sbuf_tensor("lss", [BFULL, DC], F32))
        nrm = ctx.enter_context(nc.sbuf_tensor("nrm", [BFULL, DC], F32))
        rn = ctx.enter_context(nc.sbuf_tensor("rn", [BFULL, DC], F32))
        et = ctx.enter_context(nc.sbuf_tensor("et", [BFULL, DC], F32))
        fac = ctx.enter_context(nc.sbuf_tensor("fac", [BFULL, DC], F32))
        ot = ctx.enter_context(nc.sbuf_tensor("ot", [BFULL, DIC], F32))
        warm = ctx.enter_context(nc.sbuf_tensor("warm", [1, 2], F32))
        sem_wa = ctx.enter_context(nc.semaphore("sem_wa"))
        sem_wlo = ctx.enter_context(nc.semaphore("sem_wlo"))
        sem_u = [ctx.enter_context(nc.semaphore(f"sem_u{q}")) for q in range(4)]
        sem_sq = [ctx.enter_context(nc.semaphore(f"sem_sq{q}")) for q in range(4)]
        sem_pe = ctx.enter_context(nc.semaphore("sem_pe"))
        sem_nm = ctx.enter_context(nc.semaphore("sem_nm"))
        sem_rn = ctx.enter_context(nc.semaphore("sem_rn"))
        sem_act = ctx.enter_context(nc.semaphore("sem_act"))
        sem_fp = ctx.enter_context(nc.semaphore("sem_fp"))
        sem_fin = ctx.enter_context(nc.semaphore("sem_fin"))
        sem_out = ctx.enter_context(nc.semaphore("sem_out"))

        one = nc.const_aps.tensor(1.0, (1, 1), F32)

        with nc.Block() as block:

            def u_dma(eng, q, sem):
                eng.dma_start(
                    u_all[:, q * QC * BFULL : (q + 1) * QC * BFULL],
                    bass.AP(u_t, q * QC * BFULL, [[UCOLS, P], [1, QC * BFULL]]),
                ).then_inc(sem, 16)

            @block.sync
            def _(sync):
                # W in two pieces on SP so the first scale fires early; U
                # quarters interleave ACT/Pool so consecutive-consumed
                # pieces never queue behind each other (PE eats a quarter
                # in ~0.5us; one queue delivers only every ~1.1us)
                sync.dma_start(
                    wb[:, :OFF_WLO],
                    bass.AP(wb_t, 0, [[WB_COLS, P], [1, OFF_WLO]]),
                ).then_inc(sem_wa, 16)
                sync.dma_start(
                    wb[:, OFF_WLO:],
                    bass.AP(wb_t, OFF_WLO, [[WB_COLS, P], [1, WHALF]]),
                ).then_inc(sem_wlo, 16)
                sync.wait_ge(sem_fin, 1)
                sync.dma_start(out[:, :], ot[:]).then_inc(sem_out, 16)

            @block.scalar
            def _(scalar):
                u_dma(scalar, 3, sem_u[3])
                u_dma(scalar, 1, sem_u[1])
                # warm the ln/exp ACT table (1.3us load) while DMAs stream
                scalar.activation(out=warm[:, 0:1], in_=one, func=AF.Ln)
                scalar.activation(out=warm[:, 1:2], in_=one, func=AF.Exp)
                # epilogue: ss[b,t] = sum_i S[b,t,i]^2 straight from PSUM
                # (Square is in every ACT table; accum_out does the sum)
                scalar.wait_ge(sem_pe, 1)
                s3a = ps[:].rearrange("b (t i) -> b t i", i=DD)
                for t in range(DC):
                    scalar.activation(
                        out=sq[:, t * DD : (t + 1) * DD],
                        in_=s3a[:, t],
                        func=AF.Square,
                        accum_out=ss[:, t : t + 1],
                    )
                # norm = exp(0.5*ln(ss)); et = exp(-norm); 1/norm on DVE
                scalar.activation(out=lss[:], in_=ss[:], func=AF.Ln)
                scalar.activation(
                    out=nrm[:], in_=lss[:], func=AF.Exp, scale=0.5
                ).then_inc(sem_nm, 1)
                scalar.activation(
                    out=et[:], in_=nrm[:], func=AF.Exp, scale=-1.0
                ).then_inc(sem_act, 1)

            def scale_q(eng, q, sem):
                """(1 + B) * W over W quarter q (chunks q*QC..), fused"""
                lo = q * QC
                w_v = wb[
                    :, _wcol(lo) : _wcol(lo) + QC * DIC
                ].rearrange("p (c t i) -> p c t i", t=DC, i=DD)
                eng.scalar_tensor_tensor(
                    out=w_v,
                    in0=wb[:, OFF_B : OFF_B + BCOLS]
                    .rearrange("p (c t) -> p c t", t=DC)[:, lo : lo + QC]
                    .broadcast_to([P, QC, DC, DD]),
                    scalar=1.0,
                    in1=w_v,
                    op0=ALU.add,
                    op1=ALU.mult,
                ).then_inc(sem, 1)

            @block.gpsimd
            def _(gpsimd):
                u_dma(gpsimd, 2, sem_u[2])
                u_dma(gpsimd, 0, sem_u[0])


            @block.vector
            def _(vector):
                # quarter-granular (1+B)*W in PE consumption order: only
                # the first ~400ns scale gates the PE start
                vector.wait_ge(sem_wa, 16)
                scale_q(vector, 3, sem_sq[3])
                scale_q(vector, 2, sem_sq[2])
                vector.wait_ge(sem_wlo, 16)
                scale_q(vector, 1, sem_sq[1])
                scale_q(vector, 0, sem_sq[0])
                s3 = ps[:].rearrange("b (t i) -> b t i", i=DD)
                # 1/norm in parallel with ACT's exp(-norm)
                vector.wait_ge(sem_nm, 1)
                vector.reciprocal(out=rn[:], in_=nrm[:]).then_inc(sem_rn, 1)
                # finisher: fac' = (et - 1) * rn  ;  out = (-S) * fac'
                # (two fused ops + self-sem: DVE does not interlock RAW)
                vector.wait_ge(sem_rn, 1)
                vector.wait_ge(sem_act, 1)
                vector.scalar_tensor_tensor(
                    out=fac[:],
                    in0=et[:],
                    scalar=1.0,
                    in1=rn[:],
                    op0=ALU.subtract,
                    op1=ALU.mult,
                ).then_inc(sem_fp, 1)
                vector.wait_ge(sem_fp, 1)
                vector.scalar_tensor_tensor(
                    out=ot[:].rearrange("b (t i) -> b t i", i=DD),
                    in0=s3,
                    in1=fac[:].broadcast_to([BFULL, DC, DD]),
                    scalar=-1.0,
                    op0=ALU.mult,
                    op1=ALU.mult,
                ).then_inc(sem_fin, 1)

            @block.tensor
            def _(tensor):
                # single accumulation group over all 32 chunks, upper first
                first = True
                mm = None
                for q in (3, 2, 1, 0):
                    tensor.wait_ge(sem_sq[q], 1)
                    tensor.wait_ge(sem_u[q], 16)
                    for c in range(q * QC, (q + 1) * QC):
                        mm = tensor.matmul(
                            ps[:],
                            lhsT=u_all[:, c * BFULL : (c + 1) * BFULL],
                            rhs=wb[:, _wcol(c) : _wcol(c) + DIC],
                            start=first,
                            stop=(q == 0 and c == QC - 1),
                            skip_group_check=True,
                        )
                        first = False
                mm.then_inc(sem_pe, 1)

    return nc


_CACHE = {}


def _get_nc():
    if "nc" not in _CACHE:
        _CACHE["nc"] = build_raw()
    return _CACHE["nc"]


def prep_inputs(primary_caps, W, B):
    """Host-side layout prep + sharding + bf16 cast (no arithmetic).

    Contraction row order: chunk c holds n in [c*16, (c+1)*16); within a
    chunk, partition p = j*16 + n_local.  Core c owns digit caps
    d in {2c, 2c+1} (zeros for the 6 pad slots on cores 5-7).
    """
    U = np.asarray(primary_caps, dtype=np.float32)
    Wf = np.asarray(W, dtype=np.float32)
    Bf = np.asarray(B, dtype=np.float32).reshape(D, N)

    # U^T replicated: [p, (c b)]
    Unj = np.transpose(U, (1, 2, 0))  # n j b
    Ut = np.ascontiguousarray(
        Unj.reshape(NCHUNK, 16, DP, BFULL)
        .transpose(0, 2, 1, 3)
        .reshape(NCHUNK, P, BFULL)
        .transpose(1, 0, 2)
        .reshape(P, UCOLS)
    ).astype(ml_dtypes.bfloat16)

    # per-core W slice [p, (c, t, i)] and B slice [p, (c, t)]
    Wnj = np.transpose(Wf, (1, 3, 0, 2))  # n j d i
    Wc = (
        Wnj.reshape(NCHUNK, 16, DP, D, DD)
        .transpose(0, 2, 1, 3, 4)          # c j n_l d i
        .reshape(NCHUNK, P, D, DD)
        .transpose(1, 0, 2, 3)             # p c d i
    )
    Bn = Bf.reshape(D, NCHUNK, 16)         # d c n_l

    in_maps = []
    for core in range(NCORES):
        wt = np.zeros((P, NCHUNK, DC, DD), dtype=np.float32)
        bpt = np.zeros((16, NCHUNK, DC), dtype=np.float32)
        for t in range(DC):
            d = 2 * core + t
            if d < D:
                wt[:, :, t, :] = Wc[:, :, d, :]
                bpt[:, :, t] = Bn[d].T      # [n_l, c] -> ...
        bpm = np.broadcast_to(
            bpt.reshape(1, 16, BCOLS), (DP, 16, BCOLS)
        ).reshape(P, BCOLS)
        wbm = np.ascontiguousarray(
            np.concatenate(
                [
                    wt[:, HC:].reshape(P, WHALF),   # W_hi
                    bpm,                            # B
                    wt[:, :HC].reshape(P, WHALF),   # W_lo
                ],
                axis=1,
            )
        ).astype(ml_dtypes.bfloat16)
        in_maps.append({"u_t": Ut, "wb_t": wbm})
    return in_maps


def kernel(primary_caps, W, B):
    nc = _get_nc()
    in_maps = prep_inputs(primary_caps, W, B)
    res = run_bass_kernel_spmd(nc, in_maps, core_ids=list(range(NCORES)))
    full = np.empty((BFULL, D, DD), dtype=np.float32)
    for core in range(NCORES):
        o = np.asarray(res.results[core]["out"]).reshape(BFULL, DC, DD)
        for t in range(DC):
            d = 2 * core + t
            if d < D:
                full[:, d, :] = o[:, t, :]
    return full


# revision 79
# speedup vs baseline: 1.0348x; 1.0348x over previous
"""Trainium2 Bass kernel for nn_DigitCap (sparse_attention).

Math note: the reference's softmax is over a size-1 axis, so C == 1 exactly
and the N x N attention matrix A is dead code.  The computation collapses to

    S[b,d,i]  = sum_{n,j} (1 + B[d,n]) * W[d,n,i,j] * U[b,n,j]
    out[b,d,:] = (1 - exp(-|S|)) * S / (|S| + 1e-7)

Sharding: split by digit capsule d (2 of 10 per core, zero-padded to a
uniform 2 so the SPMD program is identical on all 8 cores).

Key implementation notes:
  * bf16 inputs (tolerance is 2e-2; bf16 keeps us ~1e-2): halves HBM
    traffic and runs the PE at 1 cycle/row instead of fp32's 4.
  * six input DMAs balanced over the three DMA queues (each dma_start
    costs ~565ns sequencer time + ~900ns completion-semaphore
    propagation; the measured fabric tops out ~190GB/s aggregate,
    ~95-125GB/s per queue): SP carries W_hi+B then W_lo, ACT and Pool
    interleave the four U quarters so consecutive-consumed pieces never
    queue behind each other (PE eats a quarter in ~0.5us; one queue
    delivers only every ~1.1us).
  * (1+B)*W runs per W-quarter on DVE so only the first ~400ns scale
    gates the PE start.
  * single PSUM accumulation group across all 32 matmuls.
  * epilogue needs sqrt AND exp, which live in different ACT tables
    (1283ns load each).  sqrt(x) = exp(0.5*ln(x)) lets the whole
    epilogue use the one natural_log+exp table, pre-warmed during the
    DMA phase; 1/norm = exp(-0.5*ln(ss)) replaces eps-add + reciprocal.
    ln(0) = -inf on the pad capsules -> NaN outputs there; host discards.
  * DVE does NOT interlock same-engine read-after-write (8-deep exec
    queue reads stale operands), so the finisher is two fused
    scalar_tensor_tensor ops with one self-semaphore between them:
        fac' = (et - 1) * rn ; out = (-S) * fac'
"""

import numpy as np
import ml_dtypes
from contextlib import ExitStack

import concourse.bass as bass
import concourse.mybir as mybir
from concourse.bass_utils import run_bass_kernel_spmd

F32 = mybir.dt.float32
BF16 = mybir.dt.bfloat16
AF = mybir.ActivationFunctionType
ALU = mybir.AluOpType
P = 128
D, DD, N, DP = 10, 16, 512, 8     # digit caps, digit dim, primary caps, primary dim
K = N * DP                         # 4096 contraction
NCHUNK = K // P                    # 32 chunks of 128 contraction rows
NCORES = 8
BFULL = 64
DC = 2                             # d's per core (8*2 = 16 slots >= 10 real)
DIC = DC * DD                      # 32 output cols per core
HC = NCHUNK // 2                   # 16
QC = NCHUNK // 4                   # 8 chunks per U quarter
WHALF = HC * DIC                   # 512 cols per W half
BCOLS = NCHUNK * DC                # 64
UCOLS = NCHUNK * BFULL             # 2048
# wb DRAM/SBUF layout: [W_hi (chunks 16..31) | B (all) | W_lo (chunks 0..15)]
WB_COLS = WHALF + BCOLS + WHALF    # 1088
OFF_B = WHALF                      # 512
OFF_WLO = WHALF + BCOLS            # 576


def _wcol(c):
    """start col of chunk c's W block inside wb"""
    return (c - HC) * DIC if c >= HC else OFF_WLO + c * DIC


def build_raw():
    nc = bass.Bass()
    u_t = nc.dram_tensor("u_t", [P, UCOLS], BF16, kind="ExternalInput")
    wb_t = nc.dram_tensor("wb_t", [P, WB_COLS], BF16, kind="ExternalInput")
    out = nc.dram_tensor("out", [BFULL, DIC], F32, kind="ExternalOutput")

    with ExitStack() as ctx:
        u_all = ctx.enter_context(nc.sbuf_tensor("u_all", [P, UCOLS], BF16))
        wb = ctx.enter_context(nc.sbuf_tensor("wb", [P, WB_COLS], BF16))
        ps = ctx.enter_context(nc.psum_tensor("ps", [BFULL, DIC], F32))
        sq = ctx.enter_context(nc.sbuf_tensor("sq", [BFULL, DIC], F32))
        ss = ctx.enter_context(nc.sbuf_tensor("ss", [BFULL, DC], F32))
        lss = ctx.enter_context(nc.sbuf_tensor("lss", [BFULL, DC], F32))
        nrm = ctx.enter_context(nc.sbuf_tensor("nrm", [BFULL, DC], F32))
        rn = ctx.enter_context(nc.sbuf_tensor("rn", [BFULL, DC], F32))
        et = ctx.enter_context(nc.sbuf_tensor("et", [BFULL, DC], F32))
        fac = ctx.enter_context(nc.sbuf_tensor("fac", [BFULL, DC], F32))
        ot = ctx.enter_context(nc.sbuf_tensor("ot", [BFULL, DIC], F32))
        warm = ctx.enter_context(nc.sbuf_tensor("warm", [1, 2], F32))
        sem_wa = ctx.enter_context(nc.semaphore("sem_wa"))
        sem_wlo = ctx.enter_context(nc.semaphore("sem_wlo"))
        sem_u = [ctx.enter_context(nc.semaphore(f"sem_u{q}")) for q in range(4)]
        sem_sq = [ctx.enter_context(nc.semaphore(f"sem_sq{q}")) for q in range(4)]
        sem_pe = ctx.enter_context(nc.semaphore("sem_pe"))
        sem_nm = ctx.enter_context(nc.semaphore("sem_nm"))
        sem_rn = ctx.enter_context(nc.semaphore("sem_rn"))
        sem_act = ctx.enter_context(nc.semaphore("sem_act"))
        sem_fp = ctx.enter_context(nc.semaphore("sem_fp"))
        sem_fin = ctx.enter_context(nc.semaphore("sem_fin"))
        sem_out = ctx.enter_context(nc.semaphore("sem_out"))

        one = nc.const_aps.tensor(1.0, (1, 1), F32)

        with nc.Block() as block:

            def u_dma(eng, q, sem):
                eng.dma_start(
                    u_all[:, q * QC * BFULL : (q + 1) * QC * BFULL],
                    bass.AP(u_t, q * QC * BFULL, [[UCOLS, P], [1, QC * BFULL]]),
                ).then_inc(sem, 16)

            @block.sync
            def _(sync):
                # W in two pieces on SP so the first scale fires early; U
                # quarters interleave ACT/Pool so consecutive-consumed
                # pieces never queue behind each other (PE eats a quarter
                # in ~0.5us; one queue delivers only every ~1.1us)
                sync.dma_start(
                    wb[:, :OFF_WLO],
                    bass.AP(wb_t, 0, [[WB_COLS, P], [1, OFF_WLO]]),
                ).then_inc(sem_wa, 16)
                u_dma(sync, 1, sem_u[1])
                sync.wait_ge(sem_fin, 1)
                sync.dma_start(out[:, :], ot[:]).then_inc(sem_out, 16)

            @block.scalar
            def _(scalar):
                scalar.dma_start(
                    wb[:, OFF_WLO:],
                    bass.AP(wb_t, OFF_WLO, [[WB_COLS, P], [1, WHALF]]),
                ).then_inc(sem_wlo, 16)
                u_dma(scalar, 2, sem_u[2])
                # warm the ln/exp ACT table (1.3us load) while DMAs stream
                scalar.activation(out=warm[:, 0:1], in_=one, func=AF.Ln)
                scalar.activation(out=warm[:, 1:2], in_=one, func=AF.Exp)
                # epilogue: ss[b,t] = sum_i S[b,t,i]^2 straight from PSUM
                # (Square is in every ACT table; accum_out does the sum)
                scalar.wait_ge(sem_pe, 1)
                s3a = ps[:].rearrange("b (t i) -> b t i", i=DD)
                for t in range(DC):
                    scalar.activation(
                        out=sq[:, t * DD : (t + 1) * DD],
                        in_=s3a[:, t],
                        func=AF.Square,
                        accum_out=ss[:, t : t + 1],
                    )
                # norm = exp(0.5*ln(ss)); et = exp(-norm); 1/norm on DVE
                scalar.activation(out=lss[:], in_=ss[:], func=AF.Ln)
                scalar.activation(
                    out=nrm[:], in_=lss[:], func=AF.Exp, scale=0.5
                ).then_inc(sem_nm, 1)
                scalar.activation(
                    out=et[:], in_=nrm[:], func=AF.Exp, scale=-1.0
                ).then_inc(sem_act, 1)

            def scale_q(eng, q, sem):
                """(1 + B) * W over W quarter q (chunks q*QC..), fused"""
                lo = q * QC
                w_v = wb[
                    :, _wcol(lo) : _wcol(lo) + QC * DIC
                ].rearrange("p (c t i) -> p c t i", t=DC, i=DD)
                eng.scalar_tensor_tensor(
                    out=w_v,
                    in0=wb[:, OFF_B : OFF_B + BCOLS]
                    .rearrange("p (c t) -> p c t", t=DC)[:, lo : lo + QC]
                    .broadcast_to([P, QC, DC, DD]),
                    scalar=1.0,
                    in1=w_v,
                    op0=ALU.add,
                    op1=ALU.mult,
                ).then_inc(sem, 1)

            @block.gpsimd
            def _(gpsimd):
                u_dma(gpsimd, 3, sem_u[3])
                u_dma(gpsimd, 0, sem_u[0])


            @block.vector
            def _(vector):
                # quarter-granular (1+B)*W in PE consumption order: only
                # the first ~400ns scale gates the PE start
                vector.wait_ge(sem_wa, 16)
                scale_q(vector, 3, sem_sq[3])
                scale_q(vector, 2, sem_sq[2])
                vector.wait_ge(sem_wlo, 16)
                scale_q(vector, 1, sem_sq[1])
                scale_q(vector, 0, sem_sq[0])
                s3 = ps[:].rearrange("b (t i) -> b t i", i=DD)
                # 1/norm in parallel with ACT's exp(-norm)
                vector.wait_ge(sem_nm, 1)
                vector.reciprocal(out=rn[:], in_=nrm[:]).then_inc(sem_rn, 1)
                # finisher: fac' = (et - 1) * rn  ;  out = (-S) * fac'
                # (two fused ops + self-sem: DVE does not interlock RAW)
                vector.wait_ge(sem_rn, 1)
                vector.wait_ge(sem_act, 1)
                vector.scalar_tensor_tensor(
                    out=fac[:],
                    in0=et[:],
                    scalar=1.0,
                    in1=rn[:],
                    op0=ALU.subtract,
                    op1=ALU.mult,
                ).then_inc(sem_fp, 1)
                vector.wait_ge(sem_fp, 1)
                vector.scalar_tensor_tensor(
                    out=ot[:].rearrange("b (t i) -> b t i", i=DD),
                    in0=s3,
                    in1=fac[:].broadcast_to([BFULL, DC, DD]),
                    scalar=-1.0,
                    op0=ALU.mult,
                    op1=ALU.mult,
                ).then_inc(sem_fin, 1)

            @block.tensor
            def _(tensor):
                # single accumulation group over all 32 chunks, upper first
                first = True
                mm = None
                for q in (3, 2, 1, 0):
                    tensor.wait_ge(sem_sq[q], 1)
                    tensor.wait_ge(sem_u[q], 16)
                    for c in range(q * QC, (q + 1) * QC):
                        mm = tensor.matmul(
                            ps[:],
                            lhsT=u_all[:, c * BFULL : (c + 1) * BFULL],
                            rhs=wb[:, _wcol(c) : _wcol(c) + DIC],
                            start=first,
                            stop=(q == 0 and c == QC - 1),
                            skip_group_check=True,
                        )
                        first = False
                mm.then_inc(sem_pe, 1)

    return nc


_CACHE = {}


def _get_nc():
    if "nc" not in _CACHE:
        _CACHE["nc"] = build_raw()
    return _CACHE["nc"]


def prep_inputs(primary_caps, W, B):
    """Host-side layout prep + sharding + bf16 cast (no arithmetic).

    Contraction row order: chunk c holds n in [c*16, (c+1)*16); within a
    chunk, partition p = j*16 + n_local.  Core c owns digit caps
    d in {2c, 2c+1} (zeros for the 6 pad slots on cores 5-7).
    """
    U = np.asarray(primary_caps, dtype=np.float32)
    Wf = np.asarray(W, dtype=np.float32)
    Bf = np.asarray(B, dtype=np.float32).reshape(D, N)

    # U^T replicated: [p, (c b)]
    Unj = np.transpose(U, (1, 2, 0))  # n j b
    Ut = np.ascontiguousarray(
        Unj.reshape(NCHUNK, 16, DP, BFULL)
        .transpose(0, 2, 1, 3)
        .reshape(NCHUNK, P, BFULL)
        .transpose(1, 0, 2)
        .reshape(P, UCOLS)
    ).astype(ml_dtypes.bfloat16)

    # per-core W slice [p, (c, t, i)] and B slice [p, (c, t)]
    Wnj = np.transpose(Wf, (1, 3, 0, 2))  # n j d i
    Wc = (
        Wnj.reshape(NCHUNK, 16, DP, D, DD)
        .transpose(0, 2, 1, 3, 4)          # c j n_l d i
        .reshape(NCHUNK, P, D, DD)
        .transpose(1, 0, 2, 3)             # p c d i
    )
    Bn = Bf.reshape(D, NCHUNK, 16)         # d c n_l

    in_maps = []
    for core in range(NCORES):
        wt = np.zeros((P, NCHUNK, DC, DD), dtype=np.float32)
        bpt = np.zeros((16, NCHUNK, DC), dtype=np.float32)
        for t in range(DC):
            d = 2 * core + t
            if d < D:
                wt[:, :, t, :] = Wc[:, :, d, :]
                bpt[:, :, t] = Bn[d].T      # [n_l, c] -> ...
        bpm = np.broadcast_to(
            bpt.reshape(1, 16, BCOLS), (DP, 16, BCOLS)
        ).reshape(P, BCOLS)
        wbm = np.ascontiguousarray(
            np.concatenate(
                [
                    wt[:, HC:].reshape(P, WHALF),   # W_hi
                    bpm,                            # B
                    wt[:, :HC].reshape(P, WHALF),   # W_lo
                ],
                axis=1,
            )
        ).astype(ml_dtypes.bfloat16)
        in_maps.append({"u_t": Ut, "wb_t": wbm})
    return in_maps


def kernel(primary_caps, W, B):
    nc = _get_nc()
    in_maps = prep_inputs(primary_caps, W, B)
    res = run_bass_kernel_spmd(nc, in_maps, core_ids=list(range(NCORES)))
    full = np.empty((BFULL, D, DD), dtype=np.float32)
    for core in range(NCORES):
        o = np.asarray(res.results[core]["out"]).reshape(BFULL, DC, DD)
        for t in range(DC):
            d = 2 * core + t
            if d < D:
                full[:, d, :] = o[:, t, :]
    return full
